# revision 39
# baseline (speedup 1.0000x reference)
"""Trainium2 Bass kernel for nn_ConvSparseKernel (sparse-tap conv, 5 taps).

Computation (per reference):
    Wn[k] = row-standardized W[k]  (per (k, out) row: subtract mean over in,
            then L2-normalize)
    y[b, :, oh, ow] = (sum_k Wn[k] @ x[b, :, oh+kh_k, ow+kw_k] + bias) * NF

Shapes (full): x [16, 256, 64, 64] f32, W [5, 256, 256] f32, bias [256] f32
Output: [16, 256, 62, 62] f32.

Sharding: data-parallel over batch -- 8 cores x 2 batches each; W/bias
replicated.  The f32 gate is global-scaled rel-err < 2e-2, so the
device datapath runs in fp16 (x, W shipped as fp16 from the host; y
drained as fp16 and cast back) -- measured end-to-end error ~4e-4.
Standardization still runs on-device in f32 from the fp16 W.

Cost-model facts this schedule is built around (from TimelineSim traces):
  - fp16 matmuls cost the same 1 cycle/row as f32r (output free-size >=
    256), so fp16 does not change the 64us PE floor -- but it halves
    every DMA byte count, which is what removed the f32 version's
    6.5us of mid-start PE starvation and half the drain tail.
  - All DMA copies serialize on ONE ~360B/ns engine; HWDGE descriptor
    generation is a second serial resource shared by the SP/ACT rings
    (~0.63us per DMA); the gpsimd SWDGE generator is separate.  Every
    DMA pays a 900ns completion-semaphore before its data is usable.
  - First-byte-usable latency for any DMA is ~2.7us (ring gen + engine
    delay + copy + completion sem), so the first real matmul cannot
    start before ~4us; dummy transposes keep the PE p-state ramp warm
    until then.
  - Dependency tracking is TILE-granular against all program-order-
    prior writers, so wraw/wn/wt are split per tap and x is loaded into
    separate row-piece tiles (2-row halo) -- otherwise every matmul
    waits for the LAST x piece of its input tile.
  - The Tile scheduler's internal sim has no real DMA latencies, so ops
    gated on late data (taps 1-4 stats, oc1 stats, b1 loads) carry
    tile_wait_until timestamps to keep them from being ordered ahead of
    critical ops on the in-order engines.

Main loop: per (b, oc, 8-row chunk) one PSUM bank accumulates 10 fp16
matmuls (N=496 -> 1 cycle/row); ACT applies bias*NF + scale writing an
fp16 tile; y drains on the gpsimd ring, except the last two chunks
which use the faster ACT/SP HWDGE rings to shorten the serial tail.
"""

import os

import numpy as np

KERNEL_KEYS = ((0, 0), (0, 2), (1, 1), (2, 0), (2, 2))
IN_CH = 256
OUT_CH = 256
H = 64
OH = 62
B_FULL = 16
N_CORES = 8
B_LOCAL = B_FULL // N_CORES
NF = float(1.0 / np.sqrt(IN_CH * len(KERNEL_KEYS) + 1))
ROW_CHUNK = 8  # rows of output per PSUM tile -> N = 8*62 = 496 <= 512

# x row-piece tiles (2-row halo so chunk c reads exactly one piece):
# b0 streams ahead of the matmuls with small first pieces; b1 is bulk.
XP_B0 = ((0, 10), (8, 18), (16, 34), (32, 50), (48, 58), (56, 64))
XP_B1 = ((0, 18), (16, 34), (32, 50), (48, 64))
# chunk -> piece index for each split
CMAP_B0 = (0, 1, 2, 2, 3, 3, 4, 5)
CMAP_B1 = (0, 0, 1, 1, 2, 2, 3, 3)

# Dummy-matmul warmup calibration (see _emit).  PE SEQ issues these at
# ~233ns each, so the count directly sets when the real stream can start.
DUM_BIG = int(os.environ.get("DUM_BIG", "12"))
DUM_SPLICE = int(os.environ.get("DUM_SPLICE", "2"))
# tile_wait_until logical delays (ms): taps 1-4 stats / b1 loads / oc1 stats
WAIT_T14 = float(os.environ.get("WAIT_T14", "0.0035"))
WAIT_TAP_STEP = float(os.environ.get("WAIT_TAP_STEP", "0.0011"))
WAIT_P2 = float(os.environ.get("WAIT_P2", "0.004"))
WAIT_B1 = float(os.environ.get("WAIT_B1", "0.011"))
WAIT_OC1 = float(os.environ.get("WAIT_OC1", "0.013"))
# rows in the final drained piece (smaller -> shorter serial tail)
TAIL_ROWS = int(os.environ.get("TAIL_ROWS", "2"))

_compiled_nc = None


def _emit(tc, nc, y, x, w, bias):
    import concourse.mybir as mybir
    from concourse.masks import make_identity

    f32 = mybir.dt.float32
    f16 = mybir.dt.float16
    AF = mybir.ActivationFunctionType
    AX = mybir.AxisListType
    NTAP = len(KERNEL_KEYS)

    w_okI = w.rearrange("k o i -> o k i")
    bias2d = bias.rearrange("(p u) -> p u", u=1)

    with tc.tile_pool(name="const", bufs=1) as cpool, \
         tc.tile_pool(name="wprep", bufs=1) as wpool, \
         tc.tile_pool(name="tpsum", bufs=2, space="PSUM") as tpool, \
         tc.tile_pool(name="mmpsum", bufs=5, space="PSUM") as mpool, \
         tc.tile_pool(name="outp", bufs=12) as opool:

        # ---- SBUF tiles (split per tap / per x-piece; see module doc) --
        junk = cpool.tile([64, 64], f32, name="junk")
        ident_f32 = cpool.tile([128, 128], f32, name="ident_f32")
        ident = cpool.tile([128, 128], f16, name="ident")
        sqrt_warm = cpool.tile([64, 1], f32, name="sqrt_warm")
        # wraw[0] split: tap0 alone (critical path), taps 1-4 batched.
        wraw0a = cpool.tile([128, IN_CH], f16, name="wraw0a")
        # taps 1-4 as separate tiles/DMAs: each tap's stats start as soon
        # as its own slice lands instead of waiting for the whole block
        wraw0b = [cpool.tile([128, IN_CH], f16, name=f"wraw0b_{k}",
                             tag=f"wraw0b_{k}") for k in range(1, NTAP)]
        wraw1 = cpool.tile([128, NTAP, IN_CH], f16, name="wraw1")
        braw = [cpool.tile([128, 1], f32, name=f"braw_{oc}",
                           tag=f"braw_{oc}") for oc in range(2)]
        bnf = [cpool.tile([128, 1], f32, name=f"bnf_{oc}", tag=f"bnf_{oc}")
               for oc in range(2)]

        def wraw_ap(oc, k):
            if oc == 1:
                return wraw1[:, k, :]
            return wraw0a if k == 0 else wraw0b[k - 1]

        wn = [[wpool.tile([128, IN_CH], f16, name=f"wn_{oc}_{k}",
                          tag=f"wn_{oc}_{k}") for k in range(NTAP)]
              for oc in range(2)]
        wt = [[cpool.tile([128, 2, 128], f16, name=f"wt_{oc}_{k}",
                          tag=f"wt_{oc}_{k}") for k in range(NTAP)]
              for oc in range(2)]
        xp = [[[cpool.tile([128, r1 - r0, H], f16,
                           name=f"xp_{b}_{cc}_{pi}", tag=f"xp_{b}_{cc}_{pi}")
                for pi, (r0, r1) in enumerate(XP_B0 if b == 0 else XP_B1)]
               for cc in range(2)] for b in range(B_LOCAL)]
        st = {}
        for oc in range(2):
            for nm in ("ssq", "sums", "mu", "musums", "var", "sd", "inv"):
                st[(oc, nm)] = wpool.tile([128, NTAP], f32,
                                          name=f"{nm}_{oc}",
                                          tag=f"{nm}_{oc}")
            st[(oc, "sqs")] = wpool.tile([128, IN_CH], f32,
                                         name=f"sqs_{oc}", tag=f"sqs_{oc}")

        def xdma(eng, b, cc, pi):
            rows = (XP_B0 if b == 0 else XP_B1)[pi]
            eng.dma_start(out=xp[b][cc][pi],
                          in_=x[b, cc * 128:(cc + 1) * 128,
                                rows[0]:rows[1], :])

        # ---- early DMA issue: the copy queue drains roughly in ring-gen
        # completion order, so this interleaving is the supply schedule.
        # wraw0a leads on the SP ring (fastest seq+HWDGE chain, usable
        # ~3.0us -- it gates the whole weight-prep critical path); the
        # first cc1 x pieces ride the otherwise-idle DVE ring so the ACT
        # ring's seq only carries the table loads + stats before the cc1
        # bulk; wraw0b leads the Pool/SWDGE queue.  junk is memset on DVE
        # so the warmup dummies don't wait behind Pool descriptor gen.
        nc.sync.dma_start(out=wraw0a, in_=w_okI[0:128, 0, :])
        nc.vector.memset(junk, 1.0)
        nc.scalar.sqrt(sqrt_warm, junk[:, 0:1])   # ACT table preload
        # make_identity leads the Pool queue: it feeds the ident cast that
        # gates tap0's transpose (the first-matmul critical path), and the
        # wraw0b SWDGE gens behind it each cost ~1us of Pool time.
        make_identity(nc, ident_f32)
        xdma(nc.sync, 0, 0, 0)
        xdma(nc.sync, 0, 0, 1)
        for k in range(1, NTAP):
            nc.gpsimd.dma_start(out=wraw0b[k - 1], in_=w_okI[0:128, k, :])
        xdma(nc.sync, 0, 1, 0)
        xdma(nc.sync, 0, 1, 1)
        nc.gpsimd.dma_start(out=braw[0], in_=bias2d[0:128])
        xdma(nc.sync, 0, 0, 2)

        # ---- PE warmup: dummy matmuls on the memset scratch keep PE
        # continuously busy from ~0.4us so the p-state ramp is done when
        # the real stream starts.  (Matmuls, not transposes: CoreSim
        # validates transpose ifmaps as permutation matrices, and a real
        # identity would add a make_identity dependency that delays the
        # warmup several us behind the Pool engine's DMA descriptor gens.)
        def dummy(n):
            for _ in range(n):
                dt_ = tpool.tile([1, 64], f32, name="dum", tag="dum",
                                 bufs=1)
                nc.tensor.matmul(dt_, junk[:, 0:1], junk, start=True,
                                 stop=True)

        dummy(DUM_BIG)

        # ---- weight standardization (per tap) ----
        # ||w - mu||^2 = ssq - sums^2/N, so sq/ssq don't wait on the mean,
        # and 1/sd comes from one ACT Rsqrt (no sqrt+reciprocal pair).
        def stats_tap(oc, k, mid=None):
            ks = slice(k, k + 1)
            wsrc = wraw_ap(oc, k)
            nc.scalar.activation(st[(oc, "sqs")], wsrc,
                                 AF.Square, accum_out=st[(oc, "ssq")][:, ks])
            nc.vector.reduce_sum(out=st[(oc, "sums")][:, ks],
                                 in_=wsrc, axis=AX.X)
            nc.vector.tensor_scalar_mul(st[(oc, "mu")][:, ks],
                                        st[(oc, "sums")][:, ks], 1.0 / IN_CH)
            # musums = sums^2/N in one fused DVE op
            nc.vector.tensor_scalar(
                out=st[(oc, "musums")][:, ks], in0=st[(oc, "sums")][:, ks],
                scalar1=st[(oc, "sums")][:, ks], scalar2=1.0 / IN_CH,
                op0=mybir.AluOpType.mult, op1=mybir.AluOpType.mult)
            nc.vector.tensor_sub(out=st[(oc, "var")][:, ks],
                                 in0=st[(oc, "ssq")][:, ks],
                                 in1=st[(oc, "musums")][:, ks])
            if mid is not None:
                mid()  # DVE filler that runs under the ACT sqrt below
            nc.scalar.sqrt(st[(oc, "sd")][:, ks], st[(oc, "var")][:, ks])
            nc.vector.reciprocal(st[(oc, "inv")][:, ks],
                                 st[(oc, "sd")][:, ks])
            # wn_k = (w_k - mu_k) * inv_k, one fused DVE op
            nc.vector.tensor_scalar(
                out=wn[oc][k], in0=wsrc,
                scalar1=st[(oc, "mu")][:, ks],
                scalar2=st[(oc, "inv")][:, ks],
                op0=mybir.AluOpType.subtract,
                op1=mybir.AluOpType.mult)

        def transpose_tap(oc, k):
            for ic in range(2):
                pt = tpool.tile([128, 128], f16, name="pt", tag="pt")
                nc.tensor.transpose(
                    pt, wn[oc][k][:, ic * 128:(ic + 1) * 128], ident)
                # alternate PSUM->SBUF copy engine: DVE / ACT
                if ic == 0:
                    nc.vector.tensor_copy(out=wt[oc][k][:, ic, :], in_=pt)
                else:
                    nc.scalar.copy(wt[oc][k][:, ic, :], pt)

        # ---- main-loop helpers (per-chunk PSUM accumulation) ----
        chunk_ps = {}
        chunk_cnt = {}

        def mm(b, oc, c, k, ic, r0=None, nr=None, key=None):
            if r0 is None:
                r0 = c * ROW_CHUNK
                nr = min(ROW_CHUNK, OH - r0)
            key = key or (b, oc, c)
            if key not in chunk_ps:
                chunk_ps[key] = mpool.tile([128, nr, OH], f32, name="ps",
                                           tag="ps")
                chunk_cnt[key] = 0
            idx = chunk_cnt[key]
            kh, kw = KERNEL_KEYS[k]
            pi = (CMAP_B0 if b == 0 else CMAP_B1)[c]
            off = (XP_B0 if b == 0 else XP_B1)[pi][0]
            lr = kh + r0 - off
            rhs = xp[b][ic][pi][:, lr:lr + nr, kw:kw + OH]
            nc.tensor.matmul(chunk_ps[key], wt[oc][k][:, ic, :], rhs,
                             start=(idx == 0), stop=(idx == 2 * NTAP - 1))
            chunk_cnt[key] = idx + 1

        def drain_chunk(b, oc, c, ring=None, r0=None, nr=None, key=None):
            if r0 is None:
                r0 = c * ROW_CHUNK
                nr = min(ROW_CHUNK, OH - r0)
            key = key or (b, oc, c)
            assert chunk_cnt[key] == 2 * NTAP
            ot = opool.tile([128, nr, OH], f16, name="ot", tag="ot")
            nc.scalar.activation(ot, chunk_ps[key], AF.Identity,
                                 bias=bnf[oc], scale=NF)
            eng = ring or nc.gpsimd
            eng.dma_start(
                out=y[b, oc * 128:(oc + 1) * 128, r0:r0 + nr, :], in_=ot)
            del chunk_ps[key], chunk_cnt[key]

        def conv_chunk(b, oc, c, ring=None):
            for k in range(NTAP):
                for ic in range(2):
                    mm(b, oc, c, k, ic)
            drain_chunk(b, oc, c, ring=ring)

        NCH = (OH + ROW_CHUNK - 1) // ROW_CHUNK  # 8 chunks (last is 6 rows)

        # ---- prep0: oc0 stats/transposes fused with the first chunks'
        # matmuls.  Tap blocks cover c0/c1; the transpose for tap k+1 is
        # tucked inside tap k's block so its PSUM->SBUF copy hides under
        # matmuls.  c2 runs as one block at the end.
        # tap0 stats with the ident cast tucked into the DVE slot under the
        # ACT sqrt (ident is only needed by transpose_tap right after).
        stats_tap(0, 0,
                  mid=lambda: nc.vector.tensor_copy(out=ident, in_=ident_f32))
        # tap0's transpose + PSUM->SBUF copies BEFORE the taps 1-4 block:
        # they gate the first real matmul, and emitting them first keeps
        # the scheduler from queueing taps 1-4 stats ahead of them on the
        # in-order DVE/ACT engines.
        transpose_tap(0, 0)
        # cc1p2's 819ns copy is not consumed until ~10.5us; the logical
        # delay keeps it from hijacking the copy engine ahead of the
        # early cc0/cc1 pieces and weight slices.  (Scalar ring: nothing
        # behind it is needed early.)
        with tc.tile_wait_until(WAIT_P2):
            xdma(nc.scalar, 0, 1, 2)
        # Taps 1-4 data lands ~3.6-6us (per-tap slices): each tap's stats
        # AND transpose/copies are emitted together under a staggered
        # logical delay, so the in-order DVE/ACT queues interleave
        # ssq_k..wn_k..ptcopy_k per tap instead of running all stats
        # before any copy (which starves the early matmul stream).
        for k in range(1, NTAP):
            with tc.tile_wait_until(WAIT_T14 + (k - 1) * WAIT_TAP_STEP):
                stats_tap(0, k)
                transpose_tap(0, k)
        dummy(DUM_SPLICE)            # covers the wt0 PSUM->SBUF copy
        xdma(nc.sync, 0, 0, 3)
        xdma(nc.scalar, 0, 1, 3)
        xdma(nc.sync, 0, 0, 4)
        xdma(nc.sync, 0, 0, 5)
        xdma(nc.sync, 0, 1, 4)
        xdma(nc.sync, 0, 1, 5)
        # W.oc1 last on the SP ring: its copy lands right after the final
        # b0 pieces without displacing them on the copy engine.
        nc.sync.dma_start(out=wraw1, in_=w_okI[128:256])
        nc.sync.dma_start(out=braw[1], in_=bias2d[128:256])
        # cc0 consumed before cc1 within each tap block so the first
        # matmuls only gate on the cc0 pieces.
        for k in range(NTAP):
            mm(0, 0, 0, k, 0)
            mm(0, 0, 1, k, 0)
            mm(0, 0, 0, k, 1)
            mm(0, 0, 1, k, 1)
        # c2 as one block, ic0 sweep then ic1 sweep
        for ic in range(2):
            for k in range(NTAP):
                mm(0, 0, 2, k, ic)
        # bnf0 on ACT before the first drain activation
        nc.scalar.mul(bnf[0], braw[0], NF)
        drain_chunk(0, 0, 0)
        drain_chunk(0, 0, 1)
        drain_chunk(0, 0, 2)
        # b1 bulk loads: logically delayed so the scheduler cannot order
        # their big copies ahead of the b0 pieces on the serial copy
        # engine (b1 is only consumed from ~36us).
        with tc.tile_wait_until(WAIT_B1):
            for pi in range(len(XP_B1)):
                xdma(nc.sync, 1, 0, pi)
            for pi in range(len(XP_B1)):
                xdma(nc.scalar, 1, 1, pi)
        nc.scalar.mul(bnf[1], braw[1], NF)

        # rest of b0.oc0, with oc1 stats/transposes tucked between chunks
        # (stats under a logical delay matching W.oc1's arrival).
        for c in range(3, NCH):
            if c - 3 < NTAP:
                with tc.tile_wait_until(WAIT_OC1):
                    stats_tap(1, c - 3)
            conv_chunk(0, 0, c)
            if c - 3 < NTAP:
                transpose_tap(1, c - 3)

        for c in range(NCH):
            conv_chunk(0, 1, c)
        for c in range(NCH):
            conv_chunk(1, 0, c)
        for c in range(NCH - 1):
            conv_chunk(1, 1, c)
        # Final chunk split in two so the bulk's ACT+DMA chain overlaps the
        # tail piece's matmuls and the serial post-PE tail is act+DGE+copy+
        # sem for only TAIL_ROWS rows.
        c = NCH - 1
        r0 = c * ROW_CHUNK
        nra = OH - r0 - TAIL_ROWS
        ka, kb = (1, 1, "ta"), (1, 1, "tb")
        for k in range(NTAP):
            for ic in range(2):
                mm(1, 1, c, k, ic, r0=r0, nr=nra, key=ka)
        # ka's DMA rides the SP ring so its descriptor gen does NOT occupy
        # ACT.SEQ between the two final activations.
        drain_chunk(1, 1, c, ring=nc.sync, r0=r0, nr=nra, key=ka)
        for k in range(NTAP):
            for ic in range(2):
                mm(1, 1, c, k, ic, r0=r0 + nra, nr=TAIL_ROWS, key=kb)
        drain_chunk(1, 1, c, ring=nc.scalar, r0=r0 + nra, nr=TAIL_ROWS,
                    key=kb)


def _build_nc():
    import concourse.mybir as mybir
    import concourse.tile as tile
    from concourse import bacc

    f32 = mybir.dt.float32
    f16 = mybir.dt.float16
    nc = bacc.Bacc("TRN2", target_bir_lowering=False, debug=False)
    x = nc.dram_tensor("x", (B_LOCAL, IN_CH, H, H), f16,
                       kind="ExternalInput").ap()
    w = nc.dram_tensor("w", (len(KERNEL_KEYS), OUT_CH, IN_CH), f16,
                       kind="ExternalInput").ap()
    bias = nc.dram_tensor("bias", (OUT_CH,), f32, kind="ExternalInput").ap()
    y = nc.dram_tensor("y", (B_LOCAL, OUT_CH, OH, OH), f16,
                       kind="ExternalOutput").ap()

    with tile.TileContext(nc) as tc:
        _emit(tc, nc, y, x, w, bias)
    nc.compile()
    return nc


def _get_nc():
    global _compiled_nc
    if _compiled_nc is None:
        _compiled_nc = _build_nc()
    return _compiled_nc


def _make_in_maps(x, W, bias):
    x = np.asarray(x, dtype=np.float16)
    W = np.asarray(W, dtype=np.float16)
    bias = np.ascontiguousarray(bias, dtype=np.float32)
    return [
        {
            "x": np.ascontiguousarray(x[i * B_LOCAL:(i + 1) * B_LOCAL]),
            "w": np.ascontiguousarray(W),
            "bias": bias,
        }
        for i in range(N_CORES)
    ]


def kernel(x, W, bias):
    from concourse import bass_utils

    nc = _get_nc()
    res = bass_utils.run_bass_kernel_spmd(
        nc, _make_in_maps(x, W, bias), core_ids=list(range(N_CORES)))
    out = np.concatenate([r["y"] for r in res.results], axis=0)
    return out.astype(np.float32)
